# revision 1
# baseline (speedup 1.0000x reference)
"""Bahdanau-style attention kernel for Trainium2 (8 NeuronCores, data-parallel over batch).

Reference computation (B=16, S=2048, H=1024, 2H=2048; note S == 2H):
    h = concat([enc, broadcast(dec)], axis=2)            # [B, S, 2H]
    h = h.reshape(B, 2H, S)   # raw reshape; since S == 2H this is an IDENTITY view
    wh = tanh(W1 @ h)         # einsum('ij,bjs->bis')    # [B, 2H, 2H]
    score = w2 @ wh           # [B, 2H]
    attn = softmax(score, axis=1)
    out = einsum('bs,bsh->bh', attn, enc)                # [B, H]

Key structural facts exploited:
  * h[b, j, s] == concat[b, j, s]: for s <  H it equals enc[b, j, s];
    for s >= H it equals dec[b, s-H], independent of j (constant column).
  * Therefore columns [H, 2H) of (W1 @ h) are rank-1: rowsum(W1)[i] * dec[b, s-H].
    Only the left half needs a real matmul: W1 [2048,2048] @ enc[b] [2048,1024].
  * score = w2 @ tanh(...), softmax over 2048, then attn @ enc[b].

Sharding: data-parallel across 8 cores on batch (2 batches per core); W1/w2
replicated. Each core runs both of its batches sequentially (PSUM pressure).
"""

import numpy as np

HIDDEN = 1024
SEQ = 2048
B_FULL = 16
N_CORES = 8
B_LOC = B_FULL // N_CORES  # 2 batches per core
P = 128
KT = SEQ // P  # 16 k-tiles
E_CHUNK = 2
W1T_BUFS = 3
FIRST_M = 3
MM_BUFS = 3

_CACHE: dict = {}


def _build_nc_legacy(mm_dtype_name: str):
    """Build + compile the per-core Bass program. mm_dtype_name in ('float32','float32r')."""
    from contextlib import ExitStack

    import concourse.bass as bass
    import concourse.tile as tile
    from concourse import bacc, mybir
    from concourse.bass import ts
    from concourse.masks import make_identity

    f32 = mybir.dt.float32
    fp32_score = mm_dtype_name.endswith("_fs")
    mmdt = getattr(mybir.dt, mm_dtype_name.replace("_fs", ""))
    scdt = f32 if fp32_score else mmdt
    AF = mybir.ActivationFunctionType

    cast_dma = mmdt != f32

    def dma_in(dst, src_ap):
        # DMA that casts (rounds) when tiles are float32r
        if cast_dma:
            return nc.gpsimd.dma_start(dst, src_ap)
        return nc.sync.dma_start(dst, src_ap)

    nc = bacc.Bacc("TRN2", target_bir_lowering=False, debug=False)

    enc = nc.dram_tensor("enc", [B_LOC, SEQ, HIDDEN], f32, kind="ExternalInput").ap()
    dec = nc.dram_tensor("dec", [B_LOC, HIDDEN], f32, kind="ExternalInput").ap()
    w1 = nc.dram_tensor("w1", [SEQ, SEQ], f32, kind="ExternalInput").ap()
    w2 = nc.dram_tensor("w2", [1, SEQ], f32, kind="ExternalInput").ap()
    out = nc.dram_tensor("out", [B_LOC, HIDDEN], f32, kind="ExternalOutput").ap()

    with tile.TileContext(nc) as tc, ExitStack() as ctx:
        const = ctx.enter_context(tc.tile_pool(name="const", bufs=1))
        e_pool = ctx.enter_context(tc.tile_pool(name="e", bufs=2))
        w1n_pool = ctx.enter_context(tc.tile_pool(name="w1n", bufs=2))
        w1t_pool = ctx.enter_context(tc.tile_pool(name="w1t", bufs=2))
        whl_pool = ctx.enter_context(tc.tile_pool(name="whl", bufs=2))
        whr_pool = ctx.enter_context(tc.tile_pool(name="whr", bufs=4))
        u_pool = ctx.enter_context(tc.tile_pool(name="u", bufs=2))
        dec_pool = ctx.enter_context(tc.tile_pool(name="decp", bufs=2))
        sm_pool = ctx.enter_context(tc.tile_pool(name="sm", bufs=1))
        at_pool = ctx.enter_context(tc.tile_pool(name="at", bufs=2))
        out_pool = ctx.enter_context(tc.tile_pool(name="outp", bufs=1))
        dram_pool = ctx.enter_context(tc.tile_pool(name="dram", bufs=2, space="DRAM"))

        mm_ps = ctx.enter_context(tc.tile_pool(name="mmps", bufs=2, space="PSUM"))
        tp_ps = ctx.enter_context(tc.tile_pool(name="tpps", bufs=2, space="PSUM"))
        sc_ps = ctx.enter_context(tc.tile_pool(name="scps", bufs=1, space="PSUM"))

        identity = const.tile([P, P], f32)
        make_identity(nc, identity)

        # w2 in partition layout: w2_sb[p, o] = w2[0, o*128+p]
        w2_sb = const.tile([P, KT], scdt)
        (nc.sync.dma_start if scdt == f32 else nc.gpsimd.dma_start)(
            w2_sb, w2[0, :].rearrange("(o p) -> p o", p=P)
        )

        for b in range(B_LOC):
            # E resident: [128(t%128), 16(t//128), 1024(s)]
            e_sb = e_pool.tile([P, KT, HIDDEN], mmdt, tag="e")
            dma_in(e_sb, enc[b].rearrange("(k p) s -> p k s", p=P))

            # dec broadcast across partitions
            dec_bc = dec_pool.tile([P, HIDDEN], f32, tag="dec")
            nc.sync.dma_start(dec_bc, dec[b : b + 1, :].to_broadcast((P, HIDDEN)))

            score_ps = sc_ps.tile([1, SEQ], f32, tag="score")

            for m in range(KT):
                # W1 row-block m (natural layout) + its rowsum
                w1n = w1n_pool.tile([P, SEQ], f32, tag="w1n")
                nc.sync.dma_start(w1n, w1[ts(m, P), :])
                u_m = u_pool.tile([P, 1], f32, tag="u")
                nc.vector.reduce_sum(u_m, w1n, axis=mybir.AxisListType.X)

                # Transpose the block on PE -> w1t[t_part, k, i]
                w1t = w1t_pool.tile([P, KT, P], mmdt, tag="w1t")
                for q in range(4):
                    tp = tp_ps.tile([P, 4 * P], f32, tag="tp")
                    for j in range(4):
                        nc.tensor.transpose(
                            tp[:, j * P : (j + 1) * P],
                            w1n[:, (4 * q + j) * P : (4 * q + j + 1) * P],
                            identity,
                        )
                    nc.vector.tensor_copy(w1t[:, 4 * q : 4 * q + 4, :], tp)

                # Left half: wh_left = tanh(W1[mblk] @ E)
                whl = whl_pool.tile([P, HIDDEN], scdt, tag="whl")
                for sc in range(2):
                    mm = mm_ps.tile([P, 512], f32, tag="mm")
                    for k in range(KT):
                        nc.tensor.matmul(
                            mm,
                            w1t[:, k, :],
                            e_sb[:, k, sc * 512 : (sc + 1) * 512],
                            start=(k == 0),
                            stop=(k == KT - 1),
                        )
                    nc.scalar.activation(whl[:, sc * 512 : (sc + 1) * 512], mm, AF.Tanh)

                # Right half: wh_right = tanh(u_m * dec)
                whr = whr_pool.tile([P, HIDDEN], scdt, tag="whr")
                nc.scalar.activation(whr, dec_bc, AF.Tanh, scale=u_m)

                # score chunk accumulation: score[s] += w2[mblk] . wh[:, s]
                for sc in range(4):
                    src = (
                        whl[:, sc * 512 : (sc + 1) * 512]
                        if sc < 2
                        else whr[:, (sc - 2) * 512 : (sc - 1) * 512]
                    )
                    nc.tensor.matmul(
                        score_ps[0:1, sc * 512 : (sc + 1) * 512],
                        w2_sb[:, m : m + 1],
                        src,
                        start=(m == 0),
                        stop=(m == KT - 1),
                    )

            # Softmax (unnormalized) on [1, 2048]
            mx = sm_pool.tile([1, 1], f32, tag="mx")
            nc.vector.reduce_max(mx, score_ps, axis=mybir.AxisListType.X)
            nmx = sm_pool.tile([1, 1], f32, tag="nmx")
            nc.vector.tensor_scalar_mul(nmx, mx, -1.0)
            pexp = sm_pool.tile([1, SEQ], f32, tag="pexp")
            sme = sm_pool.tile([1, 1], f32, tag="sme")
            nc.scalar.activation(pexp, score_ps, AF.Exp, bias=nmx, accum_out=sme)
            rcp = sm_pool.tile([1, 1], f32, tag="rcp")
            nc.vector.reciprocal(rcp, sme)

            # Transpose p_exp to partition layout via DRAM bounce
            scratch = dram_pool.tile([1, SEQ], f32, tag="scr")
            nc.sync.dma_start(scratch, pexp)
            expt = at_pool.tile([P, KT], mmdt, tag="expt")
            dma_in(expt, scratch[0, :].rearrange("(k p) -> p k", p=P))

            # out[b] = (1/Z) * p_exp @ E
            outsb = out_pool.tile([1, HIDDEN], f32, tag="outsb")
            for hc in range(2):
                fin_full = mm_ps.tile([P, 512], f32, tag="mm", name=f"fin_{b}_{hc}")
                fin = fin_full[0:1, :]
                for k in range(KT):
                    nc.tensor.matmul(
                        fin,
                        expt[:, k : k + 1],
                        e_sb[:, k, hc * 512 : (hc + 1) * 512],
                        start=(k == 0),
                        stop=(k == KT - 1),
                    )
                nc.scalar.activation(
                    outsb[:, hc * 512 : (hc + 1) * 512], fin, AF.Copy, scale=rcp
                )
            nc.sync.dma_start(out[b : b + 1, :], outsb)

    nc.compile()
    return nc




def _build_nc_bf16x3():
    """bf16 hi/lo split kernel: main matmul = 3 bf16 passes (~fp32 precision at
    1 cycle/row), score matmul in float32r, final matmul = 3 bf16 passes with
    on-chip hi/lo split of the softmax weights. W1^T, hi/lo splits and W1
    rowsums are precomputed on the host and passed as inputs."""
    from contextlib import ExitStack

    import concourse.tile as tile
    from concourse import bacc, mybir
    from concourse.bass import ts

    f32 = mybir.dt.float32
    f32r = mybir.dt.float32r
    bf16 = mybir.dt.bfloat16
    AF = mybir.ActivationFunctionType

    nc = bacc.Bacc("TRN2", target_bir_lowering=False, debug=False)

    # w1t_pack[m, t, 0:128] = bf16_hi(W1^T)[t, m*128:(m+1)*128]; [.., 128:256] = lo
    w1tp = nc.dram_tensor("w1tp", [KT, SEQ, 2 * P], bf16, kind="ExternalInput").ap()
    ehi = nc.dram_tensor("ehi", [B_LOC, SEQ, HIDDEN], bf16, kind="ExternalInput").ap()
    elo = nc.dram_tensor("elo", [B_LOC, SEQ, HIDDEN], bf16, kind="ExternalInput").ap()
    dec = nc.dram_tensor("dec", [B_LOC, HIDDEN], f32, kind="ExternalInput").ap()
    usum = nc.dram_tensor("usum", [1, SEQ], f32, kind="ExternalInput").ap()
    w2 = nc.dram_tensor("w2", [1, SEQ], f32, kind="ExternalInput").ap()
    out = nc.dram_tensor("out", [B_LOC, HIDDEN], f32, kind="ExternalOutput").ap()

    with tile.TileContext(nc) as tc, ExitStack() as ctx:
        const = ctx.enter_context(tc.tile_pool(name="const", bufs=1))
        e_pool = ctx.enter_context(tc.tile_pool(name="e", bufs=2))
        w1t_pool = ctx.enter_context(tc.tile_pool(name="w1t", bufs=W1T_BUFS))
        whl_pool = ctx.enter_context(tc.tile_pool(name="whl", bufs=2))
        whr_pool = ctx.enter_context(tc.tile_pool(name="whr", bufs=4))
        dec_pool = ctx.enter_context(tc.tile_pool(name="decp", bufs=2))
        sm_pool = ctx.enter_context(tc.tile_pool(name="sm", bufs=1))
        at_pool = ctx.enter_context(tc.tile_pool(name="at", bufs=2))
        out_pool = ctx.enter_context(tc.tile_pool(name="outp", bufs=1))
        dram_pool = ctx.enter_context(tc.tile_pool(name="dram", bufs=2, space="DRAM"))

        mm_ps = ctx.enter_context(tc.tile_pool(name="mmps", bufs=MM_BUFS, space="PSUM"))
        st_ps = ctx.enter_context(tc.tile_pool(name="stps", bufs=1, space="PSUM"))
        sc_ps = ctx.enter_context(tc.tile_pool(name="scps", bufs=1, space="PSUM"))

        # u in partition layout: u_sb[p, m] = rowsum(W1)[m*128+p]
        u_sb = const.tile([P, KT], f32)
        nc.sync.dma_start(u_sb, usum[0, :].rearrange("(m p) -> p m", p=P))
        # w2 in partition layout (rounded to f32r for the score matmul)
        w2_sb = const.tile([P, KT], f32r)
        nc.gpsimd.dma_start(w2_sb, w2[0, :].rearrange("(o p) -> p o", p=P))

        for b in range(B_LOC):
            e_hi = e_pool.tile([P, KT, HIDDEN], bf16, tag="ehi")
            nc.sync.dma_start(e_hi, ehi[b].rearrange("(k p) s -> p k s", p=P))
            e_lo = e_pool.tile([P, KT, HIDDEN], bf16, tag="elo")
            nc.sync.dma_start(e_lo, elo[b].rearrange("(k p) s -> p k s", p=P))

            dec_bc = dec_pool.tile([P, HIDDEN], f32, tag="dec")
            nc.sync.dma_start(dec_bc, dec[b : b + 1, :].to_broadcast((P, HIDDEN)))

            score_ps = sc_ps.tile([1, SEQ], f32, tag="score")

            for m in range(KT):
                # W1^T block for output rows m*128..m*128+127: [t_part, k, hi|lo]
                w1t = w1t_pool.tile([P, KT, 2 * P], bf16, tag="w1t")
                nc.sync.dma_start(w1t, w1tp[m].rearrange("(k p) i -> p k i", p=P))

                whl = whl_pool.tile([P, HIDDEN], f32r, tag="whl")
                for sc in range(2):
                    mm = mm_ps.tile([P, 512], f32, tag="mm")
                    ecol = slice(sc * 512, (sc + 1) * 512)
                    for k in range(KT):
                        nc.tensor.matmul(
                            mm, w1t[:, k, 0:P], e_hi[:, k, ecol],
                            start=(k == 0), stop=False,
                        )
                    for k in range(KT):
                        nc.tensor.matmul(
                            mm, w1t[:, k, 0:P], e_lo[:, k, ecol],
                            start=False, stop=False,
                        )
                    for k in range(KT):
                        nc.tensor.matmul(
                            mm, w1t[:, k, P : 2 * P], e_hi[:, k, ecol],
                            start=False, stop=(k == KT - 1),
                        )
                    nc.scalar.activation(whl[:, ecol], mm, AF.Tanh)

                whr = whr_pool.tile([P, HIDDEN], f32r, tag="whr")
                nc.scalar.activation(whr, dec_bc, AF.Tanh, scale=u_sb[:, m : m + 1])

                for sc in range(4):
                    src = (
                        whl[:, sc * 512 : (sc + 1) * 512]
                        if sc < 2
                        else whr[:, (sc - 2) * 512 : (sc - 1) * 512]
                    )
                    nc.tensor.matmul(
                        score_ps[0:1, sc * 512 : (sc + 1) * 512],
                        w2_sb[:, m : m + 1],
                        src,
                        start=(m == 0),
                        stop=(m == KT - 1),
                    )

            # softmax pieces on [1, 2048]
            mx = sm_pool.tile([1, 1], f32, tag="mx")
            nc.vector.reduce_max(mx, score_ps, axis=mybir.AxisListType.X)
            nmx = sm_pool.tile([1, 1], f32, tag="nmx")
            nc.vector.tensor_scalar_mul(nmx, mx, -1.0)
            pexp = sm_pool.tile([1, SEQ], f32, tag="pexp")
            sme = sm_pool.tile([1, 1], f32, tag="sme")
            nc.scalar.activation(pexp, score_ps, AF.Exp, bias=nmx, accum_out=sme)
            rcp = sm_pool.tile([1, 1], f32, tag="rcp")
            nc.vector.reciprocal(rcp, sme)

            # transpose p_exp to partition layout via DRAM bounce, then hi/lo split
            scratch = dram_pool.tile([1, SEQ], f32, tag="scr")
            nc.sync.dma_start(scratch, pexp)
            expt = at_pool.tile([P, KT], f32, tag="expt")
            nc.sync.dma_start(expt, scratch[0, :].rearrange("(k p) -> p k", p=P))
            expt_hi = at_pool.tile([P, KT], bf16, tag="expt_hi")
            nc.vector.tensor_copy(expt_hi, expt)
            expt_lo = at_pool.tile([P, KT], bf16, tag="expt_lo")
            nc.vector.tensor_tensor(
                expt_lo, expt, expt_hi, mybir.AluOpType.subtract
            )

            # out[b] = (1/Z) * p_exp @ E  (3-pass bf16 hi/lo)
            outsb = out_pool.tile([1, HIDDEN], f32, tag="outsb")
            for hc in range(2):
                fin_full = mm_ps.tile([P, 512], f32, tag="mm", name=f"fin_{b}_{hc}")
                fin = fin_full[0:1, :]
                hcol = slice(hc * 512, (hc + 1) * 512)
                for k in range(KT):
                    nc.tensor.matmul(
                        fin, expt_hi[:, k : k + 1], e_hi[:, k, hcol],
                        start=(k == 0), stop=False,
                    )
                for k in range(KT):
                    nc.tensor.matmul(
                        fin, expt_hi[:, k : k + 1], e_lo[:, k, hcol],
                        start=False, stop=False,
                    )
                for k in range(KT):
                    nc.tensor.matmul(
                        fin, expt_lo[:, k : k + 1], e_hi[:, k, hcol],
                        start=False, stop=(k == KT - 1),
                    )
                nc.scalar.activation(outsb[:, hcol], fin, AF.Copy, scale=rcp)
            nc.sync.dma_start(out[b : b + 1, :], outsb)

    nc.compile()
    return nc


def _host_prep(encoded_outputs, decoder_output, W1, w2):
    import ml_dtypes

    bf16 = ml_dtypes.bfloat16
    enc = np.ascontiguousarray(encoded_outputs, dtype=np.float32)
    dec = np.ascontiguousarray(decoder_output, dtype=np.float32)
    w1 = np.asarray(W1, dtype=np.float32)
    w2c = np.ascontiguousarray(np.asarray(w2, dtype=np.float32).reshape(1, SEQ))

    a = np.ascontiguousarray(w1.T)                       # [t, i]
    hi = a.astype(bf16)
    lo = (a - hi.astype(np.float32)).astype(bf16)
    # pack[m, t, 0:128] = hi[t, m*128:(m+1)*128]; [m, t, 128:256] = lo
    hi3 = hi.reshape(SEQ, KT, P).transpose(1, 0, 2)      # [m, t, 128]
    lo3 = lo.reshape(SEQ, KT, P).transpose(1, 0, 2)
    w1tp = np.ascontiguousarray(np.concatenate([hi3, lo3], axis=2))  # [16, 2048, 256]

    e_hi = enc.astype(bf16)
    e_lo = (enc - e_hi.astype(np.float32)).astype(bf16)
    usum = np.ascontiguousarray(w1.sum(axis=1).reshape(1, SEQ))
    return enc, dec, w2c, w1tp, e_hi, e_lo, usum




def _build_nc_opt():
    """Optimized kernel: f32r main matmul (1 pass), f32r score matmul with
    host-side bf16-split w2 (2 passes, w2 representation error ~2^-22), f32r
    final matmul. W1^T and W1 rowsums precomputed on host — no PE transposes.
    All f32r rounding happens in casting gpsimd DMAs / engine output casts."""
    from contextlib import ExitStack

    import concourse.tile as tile
    from concourse import bacc, mybir
    from concourse.bass import ts

    f32 = mybir.dt.float32
    f32r = mybir.dt.float32r
    AF = mybir.ActivationFunctionType

    nc = bacc.Bacc("TRN2", target_bir_lowering=False, debug=False)

    # w1tp[m, t, :] = W1^T[t, m*128:(m+1)*128] = W1[m*128:(m+1)*128, t]^T
    w1tp = nc.dram_tensor("w1tp", [KT, SEQ, P], f32r, kind="ExternalInput").ap()
    enc = nc.dram_tensor("enc", [B_LOC, SEQ, HIDDEN], f32r, kind="ExternalInput").ap()
    dec = nc.dram_tensor("dec", [B_LOC, HIDDEN], f32, kind="ExternalInput").ap()
    usum = nc.dram_tensor("usum", [1, SEQ], f32, kind="ExternalInput").ap()
    w2hl = nc.dram_tensor("w2hl", [1, 2 * SEQ], f32r, kind="ExternalInput").ap()
    out = nc.dram_tensor("out", [B_LOC, HIDDEN], f32, kind="ExternalOutput").ap()

    with tile.TileContext(nc) as tc, ExitStack() as ctx:
        const = ctx.enter_context(tc.tile_pool(name="const", bufs=1))
        e_pool = ctx.enter_context(tc.tile_pool(name="e", bufs=2))
        w1t_pool = ctx.enter_context(tc.tile_pool(name="w1t", bufs=W1T_BUFS))
        whl_pool = ctx.enter_context(tc.tile_pool(name="whl", bufs=2))
        whr_pool = ctx.enter_context(tc.tile_pool(name="whr", bufs=4))
        dec_pool = ctx.enter_context(tc.tile_pool(name="decp", bufs=2))
        sm_pool = ctx.enter_context(tc.tile_pool(name="sm", bufs=1))
        at_pool = ctx.enter_context(tc.tile_pool(name="at", bufs=2))
        out_pool = ctx.enter_context(tc.tile_pool(name="outp", bufs=1))
        dram_pool = ctx.enter_context(tc.tile_pool(name="dram", bufs=2, space="DRAM"))

        mm_ps = ctx.enter_context(tc.tile_pool(name="mmps", bufs=MM_BUFS, space="PSUM"))
        st_ps = ctx.enter_context(tc.tile_pool(name="stps", bufs=1, space="PSUM"))
        sc_ps = ctx.enter_context(tc.tile_pool(name="scps", bufs=1, space="PSUM"))

        u_sb = const.tile([P, KT], f32)
        nc.sync.dma_start(u_sb, usum[0, :].rearrange("(m p) -> p m", p=P))
        ones128 = const.tile([P, 1], f32)
        nc.vector.memset(ones128, 1.0)
        ones2 = const.tile([2, 1], f32)
        nc.vector.memset(ones2, 1.0)
        w2p_sb = const.tile([P, 2 * KT], f32r)
        nc.sync.dma_start(w2p_sb, w2hl[0, :].rearrange("(c p) -> p c", p=P))

        def load_w1t(m, b):
            t = w1t_pool.tile([P, KT, P], f32r, tag="w1t", name=f"w1t_{b}_{m}")
            nc.sync.dma_start(t, w1tp[m].rearrange("(k p) i -> p k i", p=P))
            return t

        def tail_softmax(b, score2_ps):
            """Row-sum + transpose to [128, 16] via 16 K=2 matmuls; exp across
            128 lanes; global max broadcast via scalar DRAM bounce."""
            mx = sm_pool.tile([1, 1], f32, tag="mx", name=f"mx_{b}")
            nc.vector.reduce_max(mx, score2_ps[0:1, :], axis=mybir.AxisListType.X)
            nmx = sm_pool.tile([1, 1], f32, tag="nmx", name=f"nmx_{b}")
            nc.vector.tensor_scalar(
                nmx, mx, -1.0, -1.0, mybir.AluOpType.mult, mybir.AluOpType.add
            )
            scrm = dram_pool.tile([1, 1], f32, tag=f"scrm{b}", name=f"scrm_{b}")
            nc.sync.dma_start(scrm, nmx)
            nmx_bc = sm_pool.tile([P, 1], f32, tag="nmxbc", name=f"nmxbc_{b}")
            nc.sync.dma_start(nmx_bc, scrm.to_broadcast((P, 1)))

            s2 = sm_pool.tile([2, SEQ], f32, tag="s2", name=f"s2_{b}")
            nc.scalar.activation(s2, score2_ps, AF.Copy)
            scoret_ps = st_ps.tile([P, KT], f32, tag="st", name=f"st_{b}")
            for c in range(KT):
                nc.tensor.matmul(
                    scoret_ps[:, c : c + 1],
                    s2[0:2, c * P : (c + 1) * P],
                    ones2,
                    start=True,
                    stop=True,
                )
            expt = at_pool.tile([P, KT], f32r, tag=f"expt{b}", name=f"expt_{b}")
            sumv = sm_pool.tile([P, 1], f32, tag="sumv", name=f"sumv_{b}")
            nc.scalar.activation(expt, scoret_ps, AF.Exp, bias=nmx_bc, accum_out=sumv)
            z_ps_full = mm_ps.tile([P, 512], f32, tag="mm", name=f"zps_{b}")
            nc.tensor.matmul(
                z_ps_full[0:1, 0:1], ones128, sumv, start=True, stop=True
            )
            rcp = sm_pool.tile([1, 1], f32, tag=f"rcp{b}", name=f"rcp_{b}")
            nc.vector.reciprocal(rcp, z_ps_full[0:1, 0:1])
            return expt, rcp

        def tail_final(b, e_sb, expt, rcp):
            outsb = out_pool.tile([1, HIDDEN], f32, tag="outsb", name=f"outsb_{b}")
            for hc in range(2):
                fin_full = mm_ps.tile([P, 512], f32, tag="mm", name=f"fin_{b}_{hc}")
                fin = fin_full[0:1, :]
                hcol = slice(hc * 512, (hc + 1) * 512)
                for k in range(KT):
                    nc.tensor.matmul(
                        fin, expt[:, k : k + 1], e_sb[:, k, hcol],
                        start=(k == 0), stop=(k == KT - 1),
                    )
                nc.scalar.activation(outsb[:, hcol], fin, AF.Copy, scale=rcp)
            nc.sync.dma_start(out[b : b + 1, :], outsb)

        def load_dec(b):
            dec_bc = dec_pool.tile([P, HIDDEN], f32, tag="dec", name=f"dec_{b}")
            nc.sync.dma_start(dec_bc, dec[b : b + 1, :].to_broadcast((P, HIDDEN)))
            return dec_bc

        def main_loop(b, e_sb, w1t_first, dec_bc):
            score2_ps = sc_ps.tile([2, SEQ], f32, tag="score", name=f"score_{b}")
            # Right half first: tanh(u x dec) needs no E -- fills the E-load window
            for m in range(KT):
                whr = whr_pool.tile([P, HIDDEN], f32r, tag="whr", name=f"whr_{b}_{m}")
                nc.scalar.activation(whr, dec_bc, AF.Tanh, scale=u_sb[:, m : m + 1])
                for sc in (2, 3):
                    nc.tensor.matmul(
                        score2_ps[0:2, sc * 512 : (sc + 1) * 512],
                        w2p_sb[:, 2 * m : 2 * m + 2],
                        whr[:, (sc - 2) * 512 : (sc - 1) * 512],
                        start=(m == 0),
                        stop=(m == KT - 1),
                    )
            # Left half: the real matmul
            w1t = w1t_first
            for m in range(KT):
                w1t_next = load_w1t(m + 1, b) if m + 1 < KT else None
                whl = whl_pool.tile([P, HIDDEN], f32r, tag="whl", name=f"whl_{b}_{m}")
                for sc in range(2):
                    mm = mm_ps.tile([P, 512], f32, tag="mm", name=f"mm_{b}_{m}_{sc}")
                    ecol = slice(sc * 512, (sc + 1) * 512)
                    for k in range(KT):
                        nc.tensor.matmul(
                            mm, w1t[:, k, :], e_sb[:, k, ecol],
                            start=(k == 0), stop=(k == KT - 1),
                        )
                    nc.scalar.activation(whl[:, ecol], mm, AF.Tanh)
                for sc in range(2):
                    nc.tensor.matmul(
                        score2_ps[0:2, sc * 512 : (sc + 1) * 512],
                        w2p_sb[:, 2 * m : 2 * m + 2],
                        whl[:, sc * 512 : (sc + 1) * 512],
                        start=(m == 0),
                        stop=(m == KT - 1),
                    )
                w1t = w1t_next
            return score2_ps

        def load_e(b):
            e_sb = e_pool.tile([P, KT, HIDDEN], f32r, tag="e", name=f"e_{b}")
            enc_t = enc[b].rearrange("(k p) s -> p k s", p=P)
            for kg in range(0, KT, E_CHUNK):
                nc.sync.dma_start(
                    e_sb[:, kg : kg + E_CHUNK, :], enc_t[:, kg : kg + E_CHUNK, :]
                )
            return e_sb

        # software-pipelined schedule: w1t[0] first so PE starts ASAP;
        # batch b's final matmuls emitted after batch b+1's main loop so they
        # fill the PE gap during b+1's softmax chain.
        dec0 = load_dec(0)
        dec1 = load_dec(1)
        w1t0_b0 = load_w1t(0, 0)
        e0 = load_e(0)
        score0 = main_loop(0, e0, w1t0_b0, dec0)
        expt0, rcp0 = tail_softmax(0, score0)
        w1t0_b1 = load_w1t(0, 1)
        e1 = load_e(1)
        score1 = main_loop(1, e1, w1t0_b1, dec1)
        tail_final(0, e0, expt0, rcp0)
        expt1, rcp1 = tail_softmax(1, score1)
        tail_final(1, e1, expt1, rcp1)

    nc.compile()
    return nc


def _host_prep_opt(encoded_outputs, decoder_output, W1, w2):
    import ml_dtypes

    bf16 = ml_dtypes.bfloat16
    enc = np.ascontiguousarray(encoded_outputs, dtype=np.float32)
    dec = np.ascontiguousarray(decoder_output, dtype=np.float32)
    w1 = np.asarray(W1, dtype=np.float32)
    w2f = np.asarray(w2, dtype=np.float32).reshape(-1)

    a = np.ascontiguousarray(w1.T)                       # [t, i]
    w1tp = np.ascontiguousarray(a.reshape(SEQ, KT, P).transpose(1, 0, 2))  # [m, t, 128]
    usum = np.ascontiguousarray(w1.sum(axis=1).reshape(1, SEQ))
    w2hi = w2f.astype(bf16).astype(np.float32)           # exactly representable
    w2lo = w2f - w2hi
    # interleave hi/lo per 128-chunk: w2hl[(2m+j)*128+p] = (hi,lo)[j][m*128+p]
    pair = np.stack([w2hi.reshape(KT, P), w2lo.reshape(KT, P)], axis=1)  # [m, 2, p]
    w2hl = np.ascontiguousarray(pair.reshape(1, 2 * SEQ))
    return enc, dec, w1tp, usum, w2hl


def _get_nc(mode: str):
    if mode not in _CACHE:
        if mode == "bf16x3":
            _CACHE[mode] = _build_nc_bf16x3()
        elif mode == "opt":
            _CACHE[mode] = _build_nc_opt()
        else:
            _CACHE[mode] = _build_nc_legacy(mode)
    return _CACHE[mode]


MM_DTYPE = "opt"


def kernel(encoded_outputs, decoder_output, W1, w2):
    from concourse.bass_utils import run_bass_kernel_spmd

    nc = _get_nc(MM_DTYPE)
    if MM_DTYPE == "opt":
        enc, dec, w1tp, usum, w2hl = _host_prep_opt(
            encoded_outputs, decoder_output, W1, w2
        )
        in_maps = [
            {
                "enc": np.ascontiguousarray(enc[i * B_LOC : (i + 1) * B_LOC]),
                "dec": np.ascontiguousarray(dec[i * B_LOC : (i + 1) * B_LOC]),
                "w1tp": w1tp,
                "usum": usum,
                "w2hl": w2hl,
            }
            for i in range(N_CORES)
        ]
    elif MM_DTYPE == "bf16x3":
        enc, dec, w2c, w1tp, e_hi, e_lo, usum = _host_prep(
            encoded_outputs, decoder_output, W1, w2
        )
        in_maps = [
            {
                "ehi": np.ascontiguousarray(e_hi[i * B_LOC : (i + 1) * B_LOC]),
                "elo": np.ascontiguousarray(e_lo[i * B_LOC : (i + 1) * B_LOC]),
                "dec": np.ascontiguousarray(dec[i * B_LOC : (i + 1) * B_LOC]),
                "w1tp": w1tp,
                "usum": usum,
                "w2": w2c,
            }
            for i in range(N_CORES)
        ]
    else:
        enc = np.ascontiguousarray(encoded_outputs, dtype=np.float32)
        dec = np.ascontiguousarray(decoder_output, dtype=np.float32)
        w1 = np.ascontiguousarray(W1, dtype=np.float32)
        w2c = np.ascontiguousarray(w2, dtype=np.float32)
        in_maps = [
            {
                "enc": np.ascontiguousarray(enc[i * B_LOC : (i + 1) * B_LOC]),
                "dec": np.ascontiguousarray(dec[i * B_LOC : (i + 1) * B_LOC]),
                "w1": w1,
                "w2": w2c,
            }
            for i in range(N_CORES)
        ]
    res = run_bass_kernel_spmd(nc, in_maps, core_ids=list(range(N_CORES)))
    return np.concatenate([r["out"] for r in res.results], axis=0)



# revision 14
# speedup vs baseline: 1.1574x; 1.1574x over previous
"""Bahdanau-style attention kernel for Trainium2 (8 NeuronCores, data-parallel over batch).

Reference computation (B=16, S=2048, H=1024, 2H=2048; note S == 2H):
    h = concat([enc, broadcast(dec)], axis=2)            # [B, S, 2H]
    h = h.reshape(B, 2H, S)   # raw reshape; since S == 2H this is an IDENTITY view
    wh = tanh(W1 @ h)         # einsum('ij,bjs->bis')    # [B, 2H, 2H]
    score = w2 @ wh           # [B, 2H]
    attn = softmax(score, axis=1)
    out = einsum('bs,bsh->bh', attn, enc)                # [B, H]

Key structural facts exploited:
  * h[b, j, s] == concat[b, j, s]: for s <  H it equals enc[b, j, s];
    for s >= H it equals dec[b, s-H], independent of j (constant column).
  * Therefore columns [H, 2H) of (W1 @ h) are rank-1: rowsum(W1)[i] * dec[b, s-H].
    Only the left half needs a real matmul: W1 [2048,2048] @ enc[b] [2048,1024].
  * score = w2 @ tanh(...), softmax over 2048, then attn @ enc[b].

Sharding: data-parallel across 8 cores on batch (2 batches per core); W1/w2
replicated. Each core runs both of its batches sequentially (PSUM pressure).
"""

import numpy as np

HIDDEN = 1024
SEQ = 2048
B_FULL = 16
N_CORES = 8
B_LOC = B_FULL // N_CORES  # 2 batches per core
P = 128
KT = SEQ // P  # 16 k-tiles
E_CHUNK = 2
W1T_BUFS = 3
FIRST_M = 3
MM_BUFS = 3

_CACHE: dict = {}


def _build_nc_legacy(mm_dtype_name: str):
    """Build + compile the per-core Bass program. mm_dtype_name in ('float32','float32r')."""
    from contextlib import ExitStack

    import concourse.bass as bass
    import concourse.tile as tile
    from concourse import bacc, mybir
    from concourse.bass import ts
    from concourse.masks import make_identity

    f32 = mybir.dt.float32
    fp32_score = mm_dtype_name.endswith("_fs")
    mmdt = getattr(mybir.dt, mm_dtype_name.replace("_fs", ""))
    scdt = f32 if fp32_score else mmdt
    AF = mybir.ActivationFunctionType

    cast_dma = mmdt != f32

    def dma_in(dst, src_ap):
        # DMA that casts (rounds) when tiles are float32r
        if cast_dma:
            return nc.gpsimd.dma_start(dst, src_ap)
        return nc.sync.dma_start(dst, src_ap)

    nc = bacc.Bacc("TRN2", target_bir_lowering=False, debug=False)

    enc = nc.dram_tensor("enc", [B_LOC, SEQ, HIDDEN], f32, kind="ExternalInput").ap()
    dec = nc.dram_tensor("dec", [B_LOC, HIDDEN], f32, kind="ExternalInput").ap()
    w1 = nc.dram_tensor("w1", [SEQ, SEQ], f32, kind="ExternalInput").ap()
    w2 = nc.dram_tensor("w2", [1, SEQ], f32, kind="ExternalInput").ap()
    out = nc.dram_tensor("out", [B_LOC, HIDDEN], f32, kind="ExternalOutput").ap()

    with tile.TileContext(nc) as tc, ExitStack() as ctx:
        const = ctx.enter_context(tc.tile_pool(name="const", bufs=1))
        e_pool = ctx.enter_context(tc.tile_pool(name="e", bufs=2))
        w1n_pool = ctx.enter_context(tc.tile_pool(name="w1n", bufs=2))
        w1t_pool = ctx.enter_context(tc.tile_pool(name="w1t", bufs=2))
        whl_pool = ctx.enter_context(tc.tile_pool(name="whl", bufs=2))
        whr_pool = ctx.enter_context(tc.tile_pool(name="whr", bufs=4))
        u_pool = ctx.enter_context(tc.tile_pool(name="u", bufs=2))
        dec_pool = ctx.enter_context(tc.tile_pool(name="decp", bufs=2))
        sm_pool = ctx.enter_context(tc.tile_pool(name="sm", bufs=1))
        at_pool = ctx.enter_context(tc.tile_pool(name="at", bufs=2))
        out_pool = ctx.enter_context(tc.tile_pool(name="outp", bufs=1))
        dram_pool = ctx.enter_context(tc.tile_pool(name="dram", bufs=2, space="DRAM"))

        mm_ps = ctx.enter_context(tc.tile_pool(name="mmps", bufs=2, space="PSUM"))
        tp_ps = ctx.enter_context(tc.tile_pool(name="tpps", bufs=2, space="PSUM"))
        sc_ps = ctx.enter_context(tc.tile_pool(name="scps", bufs=1, space="PSUM"))

        identity = const.tile([P, P], f32)
        make_identity(nc, identity)

        # w2 in partition layout: w2_sb[p, o] = w2[0, o*128+p]
        w2_sb = const.tile([P, KT], scdt)
        (nc.sync.dma_start if scdt == f32 else nc.gpsimd.dma_start)(
            w2_sb, w2[0, :].rearrange("(o p) -> p o", p=P)
        )

        for b in range(B_LOC):
            # E resident: [128(t%128), 16(t//128), 1024(s)]
            e_sb = e_pool.tile([P, KT, HIDDEN], mmdt, tag="e")
            dma_in(e_sb, enc[b].rearrange("(k p) s -> p k s", p=P))

            # dec broadcast across partitions
            dec_bc = dec_pool.tile([P, HIDDEN], f32, tag="dec")
            nc.sync.dma_start(dec_bc, dec[b : b + 1, :].to_broadcast((P, HIDDEN)))

            score_ps = sc_ps.tile([1, SEQ], f32, tag="score")

            for m in range(KT):
                # W1 row-block m (natural layout) + its rowsum
                w1n = w1n_pool.tile([P, SEQ], f32, tag="w1n")
                nc.sync.dma_start(w1n, w1[ts(m, P), :])
                u_m = u_pool.tile([P, 1], f32, tag="u")
                nc.vector.reduce_sum(u_m, w1n, axis=mybir.AxisListType.X)

                # Transpose the block on PE -> w1t[t_part, k, i]
                w1t = w1t_pool.tile([P, KT, P], mmdt, tag="w1t")
                for q in range(4):
                    tp = tp_ps.tile([P, 4 * P], f32, tag="tp")
                    for j in range(4):
                        nc.tensor.transpose(
                            tp[:, j * P : (j + 1) * P],
                            w1n[:, (4 * q + j) * P : (4 * q + j + 1) * P],
                            identity,
                        )
                    nc.vector.tensor_copy(w1t[:, 4 * q : 4 * q + 4, :], tp)

                # Left half: wh_left = tanh(W1[mblk] @ E)
                whl = whl_pool.tile([P, HIDDEN], scdt, tag="whl")
                for sc in range(2):
                    mm = mm_ps.tile([P, 512], f32, tag="mm")
                    for k in range(KT):
                        nc.tensor.matmul(
                            mm,
                            w1t[:, k, :],
                            e_sb[:, k, sc * 512 : (sc + 1) * 512],
                            start=(k == 0),
                            stop=(k == KT - 1),
                        )
                    nc.scalar.activation(whl[:, sc * 512 : (sc + 1) * 512], mm, AF.Tanh)

                # Right half: wh_right = tanh(u_m * dec)
                whr = whr_pool.tile([P, HIDDEN], scdt, tag="whr")
                nc.scalar.activation(whr, dec_bc, AF.Tanh, scale=u_m)

                # score chunk accumulation: score[s] += w2[mblk] . wh[:, s]
                for sc in range(4):
                    src = (
                        whl[:, sc * 512 : (sc + 1) * 512]
                        if sc < 2
                        else whr[:, (sc - 2) * 512 : (sc - 1) * 512]
                    )
                    nc.tensor.matmul(
                        score_ps[0:1, sc * 512 : (sc + 1) * 512],
                        w2_sb[:, m : m + 1],
                        src,
                        start=(m == 0),
                        stop=(m == KT - 1),
                    )

            # Softmax (unnormalized) on [1, 2048]
            mx = sm_pool.tile([1, 1], f32, tag="mx")
            nc.vector.reduce_max(mx, score_ps, axis=mybir.AxisListType.X)
            nmx = sm_pool.tile([1, 1], f32, tag="nmx")
            nc.vector.tensor_scalar_mul(nmx, mx, -1.0)
            pexp = sm_pool.tile([1, SEQ], f32, tag="pexp")
            sme = sm_pool.tile([1, 1], f32, tag="sme")
            nc.scalar.activation(pexp, score_ps, AF.Exp, bias=nmx, accum_out=sme)
            rcp = sm_pool.tile([1, 1], f32, tag="rcp")
            nc.vector.reciprocal(rcp, sme)

            # Transpose p_exp to partition layout via DRAM bounce
            scratch = dram_pool.tile([1, SEQ], f32, tag="scr")
            nc.sync.dma_start(scratch, pexp)
            expt = at_pool.tile([P, KT], mmdt, tag="expt")
            dma_in(expt, scratch[0, :].rearrange("(k p) -> p k", p=P))

            # out[b] = (1/Z) * p_exp @ E
            outsb = out_pool.tile([1, HIDDEN], f32, tag="outsb")
            for hc in range(2):
                fin_full = mm_ps.tile([P, 512], f32, tag="mm", name=f"fin_{b}_{hc}")
                fin = fin_full[0:1, :]
                for k in range(KT):
                    nc.tensor.matmul(
                        fin,
                        expt[:, k : k + 1],
                        e_sb[:, k, hc * 512 : (hc + 1) * 512],
                        start=(k == 0),
                        stop=(k == KT - 1),
                    )
                nc.scalar.activation(
                    outsb[:, hc * 512 : (hc + 1) * 512], fin, AF.Copy, scale=rcp
                )
            nc.sync.dma_start(out[b : b + 1, :], outsb)

    nc.compile()
    return nc




def _build_nc_bf16x3():
    """bf16 hi/lo split kernel: main matmul = 3 bf16 passes (~fp32 precision at
    1 cycle/row), score matmul in float32r, final matmul = 3 bf16 passes with
    on-chip hi/lo split of the softmax weights. W1^T, hi/lo splits and W1
    rowsums are precomputed on the host and passed as inputs."""
    from contextlib import ExitStack

    import concourse.tile as tile
    from concourse import bacc, mybir
    from concourse.bass import ts

    f32 = mybir.dt.float32
    f32r = mybir.dt.float32r
    bf16 = mybir.dt.bfloat16
    AF = mybir.ActivationFunctionType

    nc = bacc.Bacc("TRN2", target_bir_lowering=False, debug=False)

    # w1t_pack[m, t, 0:128] = bf16_hi(W1^T)[t, m*128:(m+1)*128]; [.., 128:256] = lo
    w1tp = nc.dram_tensor("w1tp", [KT, SEQ, 2 * P], bf16, kind="ExternalInput").ap()
    ehi = nc.dram_tensor("ehi", [B_LOC, SEQ, HIDDEN], bf16, kind="ExternalInput").ap()
    elo = nc.dram_tensor("elo", [B_LOC, SEQ, HIDDEN], bf16, kind="ExternalInput").ap()
    dec = nc.dram_tensor("dec", [B_LOC, HIDDEN], f32, kind="ExternalInput").ap()
    usum = nc.dram_tensor("usum", [1, SEQ], f32, kind="ExternalInput").ap()
    w2 = nc.dram_tensor("w2", [1, SEQ], f32, kind="ExternalInput").ap()
    out = nc.dram_tensor("out", [B_LOC, HIDDEN], f32, kind="ExternalOutput").ap()

    with tile.TileContext(nc) as tc, ExitStack() as ctx:
        const = ctx.enter_context(tc.tile_pool(name="const", bufs=1))
        e_pool = ctx.enter_context(tc.tile_pool(name="e", bufs=2))
        w1t_pool = ctx.enter_context(tc.tile_pool(name="w1t", bufs=W1T_BUFS))
        whl_pool = ctx.enter_context(tc.tile_pool(name="whl", bufs=2))
        whr_pool = ctx.enter_context(tc.tile_pool(name="whr", bufs=4))
        dec_pool = ctx.enter_context(tc.tile_pool(name="decp", bufs=2))
        sm_pool = ctx.enter_context(tc.tile_pool(name="sm", bufs=1))
        at_pool = ctx.enter_context(tc.tile_pool(name="at", bufs=2))
        out_pool = ctx.enter_context(tc.tile_pool(name="outp", bufs=1))
        dram_pool = ctx.enter_context(tc.tile_pool(name="dram", bufs=2, space="DRAM"))

        mm_ps = ctx.enter_context(tc.tile_pool(name="mmps", bufs=MM_BUFS, space="PSUM"))
        st_ps = ctx.enter_context(tc.tile_pool(name="stps", bufs=1, space="PSUM"))
        sc_ps = ctx.enter_context(tc.tile_pool(name="scps", bufs=1, space="PSUM"))

        # u in partition layout: u_sb[p, m] = rowsum(W1)[m*128+p]
        u_sb = const.tile([P, KT], f32)
        nc.sync.dma_start(u_sb, usum[0, :].rearrange("(m p) -> p m", p=P))
        # w2 in partition layout (rounded to f32r for the score matmul)
        w2_sb = const.tile([P, KT], f32r)
        nc.gpsimd.dma_start(w2_sb, w2[0, :].rearrange("(o p) -> p o", p=P))

        for b in range(B_LOC):
            e_hi = e_pool.tile([P, KT, HIDDEN], bf16, tag="ehi")
            nc.sync.dma_start(e_hi, ehi[b].rearrange("(k p) s -> p k s", p=P))
            e_lo = e_pool.tile([P, KT, HIDDEN], bf16, tag="elo")
            nc.sync.dma_start(e_lo, elo[b].rearrange("(k p) s -> p k s", p=P))

            dec_bc = dec_pool.tile([P, HIDDEN], f32, tag="dec")
            nc.sync.dma_start(dec_bc, dec[b : b + 1, :].to_broadcast((P, HIDDEN)))

            score_ps = sc_ps.tile([1, SEQ], f32, tag="score")

            for m in range(KT):
                # W1^T block for output rows m*128..m*128+127: [t_part, k, hi|lo]
                w1t = w1t_pool.tile([P, KT, 2 * P], bf16, tag="w1t")
                nc.sync.dma_start(w1t, w1tp[m].rearrange("(k p) i -> p k i", p=P))

                whl = whl_pool.tile([P, HIDDEN], f32r, tag="whl")
                for sc in range(2):
                    mm = mm_ps.tile([P, 512], f32, tag="mm")
                    ecol = slice(sc * 512, (sc + 1) * 512)
                    for k in range(KT):
                        nc.tensor.matmul(
                            mm, w1t[:, k, 0:P], e_hi[:, k, ecol],
                            start=(k == 0), stop=False,
                        )
                    for k in range(KT):
                        nc.tensor.matmul(
                            mm, w1t[:, k, 0:P], e_lo[:, k, ecol],
                            start=False, stop=False,
                        )
                    for k in range(KT):
                        nc.tensor.matmul(
                            mm, w1t[:, k, P : 2 * P], e_hi[:, k, ecol],
                            start=False, stop=(k == KT - 1),
                        )
                    nc.scalar.activation(whl[:, ecol], mm, AF.Tanh)

                whr = whr_pool.tile([P, HIDDEN], f32r, tag="whr")
                nc.scalar.activation(whr, dec_bc, AF.Tanh, scale=u_sb[:, m : m + 1])

                for sc in range(4):
                    src = (
                        whl[:, sc * 512 : (sc + 1) * 512]
                        if sc < 2
                        else whr[:, (sc - 2) * 512 : (sc - 1) * 512]
                    )
                    nc.tensor.matmul(
                        score_ps[0:1, sc * 512 : (sc + 1) * 512],
                        w2_sb[:, m : m + 1],
                        src,
                        start=(m == 0),
                        stop=(m == KT - 1),
                    )

            # softmax pieces on [1, 2048]
            mx = sm_pool.tile([1, 1], f32, tag="mx")
            nc.vector.reduce_max(mx, score_ps, axis=mybir.AxisListType.X)
            nmx = sm_pool.tile([1, 1], f32, tag="nmx")
            nc.vector.tensor_scalar_mul(nmx, mx, -1.0)
            pexp = sm_pool.tile([1, SEQ], f32, tag="pexp")
            sme = sm_pool.tile([1, 1], f32, tag="sme")
            nc.scalar.activation(pexp, score_ps, AF.Exp, bias=nmx, accum_out=sme)
            rcp = sm_pool.tile([1, 1], f32, tag="rcp")
            nc.vector.reciprocal(rcp, sme)

            # transpose p_exp to partition layout via DRAM bounce, then hi/lo split
            scratch = dram_pool.tile([1, SEQ], f32, tag="scr")
            nc.sync.dma_start(scratch, pexp)
            expt = at_pool.tile([P, KT], f32, tag="expt")
            nc.sync.dma_start(expt, scratch[0, :].rearrange("(k p) -> p k", p=P))
            expt_hi = at_pool.tile([P, KT], bf16, tag="expt_hi")
            nc.vector.tensor_copy(expt_hi, expt)
            expt_lo = at_pool.tile([P, KT], bf16, tag="expt_lo")
            nc.vector.tensor_tensor(
                expt_lo, expt, expt_hi, mybir.AluOpType.subtract
            )

            # out[b] = (1/Z) * p_exp @ E  (3-pass bf16 hi/lo)
            outsb = out_pool.tile([1, HIDDEN], f32, tag="outsb")
            for hc in range(2):
                fin_full = mm_ps.tile([P, 512], f32, tag="mm", name=f"fin_{b}_{hc}")
                fin = fin_full[0:1, :]
                hcol = slice(hc * 512, (hc + 1) * 512)
                for k in range(KT):
                    nc.tensor.matmul(
                        fin, expt_hi[:, k : k + 1], e_hi[:, k, hcol],
                        start=(k == 0), stop=False,
                    )
                for k in range(KT):
                    nc.tensor.matmul(
                        fin, expt_hi[:, k : k + 1], e_lo[:, k, hcol],
                        start=False, stop=False,
                    )
                for k in range(KT):
                    nc.tensor.matmul(
                        fin, expt_lo[:, k : k + 1], e_hi[:, k, hcol],
                        start=False, stop=(k == KT - 1),
                    )
                nc.scalar.activation(outsb[:, hcol], fin, AF.Copy, scale=rcp)
            nc.sync.dma_start(out[b : b + 1, :], outsb)

    nc.compile()
    return nc


def _host_prep(encoded_outputs, decoder_output, W1, w2):
    import ml_dtypes

    bf16 = ml_dtypes.bfloat16
    enc = np.ascontiguousarray(encoded_outputs, dtype=np.float32)
    dec = np.ascontiguousarray(decoder_output, dtype=np.float32)
    w1 = np.asarray(W1, dtype=np.float32)
    w2c = np.ascontiguousarray(np.asarray(w2, dtype=np.float32).reshape(1, SEQ))

    a = np.ascontiguousarray(w1.T)                       # [t, i]
    hi = a.astype(bf16)
    lo = (a - hi.astype(np.float32)).astype(bf16)
    # pack[m, t, 0:128] = hi[t, m*128:(m+1)*128]; [m, t, 128:256] = lo
    hi3 = hi.reshape(SEQ, KT, P).transpose(1, 0, 2)      # [m, t, 128]
    lo3 = lo.reshape(SEQ, KT, P).transpose(1, 0, 2)
    w1tp = np.ascontiguousarray(np.concatenate([hi3, lo3], axis=2))  # [16, 2048, 256]

    e_hi = enc.astype(bf16)
    e_lo = (enc - e_hi.astype(np.float32)).astype(bf16)
    usum = np.ascontiguousarray(w1.sum(axis=1).reshape(1, SEQ))
    return enc, dec, w2c, w1tp, e_hi, e_lo, usum




def _build_nc_opt():
    """Optimized kernel: f32r main matmul (1 pass), f32r score matmul with
    host-side bf16-split w2 (2 passes, w2 representation error ~2^-22), f32r
    final matmul. W1^T and W1 rowsums precomputed on host — no PE transposes.
    All f32r rounding happens in casting gpsimd DMAs / engine output casts."""
    from contextlib import ExitStack

    import concourse.tile as tile
    from concourse import bacc, mybir
    from concourse.bass import ts

    f32 = mybir.dt.float32
    f32r = mybir.dt.float32r
    AF = mybir.ActivationFunctionType

    nc = bacc.Bacc("TRN2", target_bir_lowering=False, debug=False)

    # w1tp[m, t, :] = W1^T[t, m*128:(m+1)*128] = W1[m*128:(m+1)*128, t]^T
    w1tp = nc.dram_tensor("w1tp", [KT, SEQ, P], f32r, kind="ExternalInput").ap()
    enc = nc.dram_tensor("enc", [B_LOC, SEQ, HIDDEN], f32r, kind="ExternalInput").ap()
    dec = nc.dram_tensor("dec", [B_LOC, HIDDEN], f32, kind="ExternalInput").ap()
    usum = nc.dram_tensor("usum", [1, SEQ], f32, kind="ExternalInput").ap()
    w2hl = nc.dram_tensor("w2hl", [1, 2 * SEQ], f32r, kind="ExternalInput").ap()
    out = nc.dram_tensor("out", [B_LOC, HIDDEN], f32, kind="ExternalOutput").ap()

    with tile.TileContext(nc) as tc, ExitStack() as ctx:
        const = ctx.enter_context(tc.tile_pool(name="const", bufs=1))
        e_pool = ctx.enter_context(tc.tile_pool(name="e", bufs=2))
        w1t_pool = ctx.enter_context(tc.tile_pool(name="w1t", bufs=W1T_BUFS))
        whl_pool = ctx.enter_context(tc.tile_pool(name="whl", bufs=2))
        whr_pool = ctx.enter_context(tc.tile_pool(name="whr", bufs=4))
        dec_pool = ctx.enter_context(tc.tile_pool(name="decp", bufs=2))
        sm_pool = ctx.enter_context(tc.tile_pool(name="sm", bufs=1))
        at_pool = ctx.enter_context(tc.tile_pool(name="at", bufs=2))
        out_pool = ctx.enter_context(tc.tile_pool(name="outp", bufs=1))
        dram_pool = ctx.enter_context(tc.tile_pool(name="dram", bufs=2, space="DRAM"))

        mm_ps = ctx.enter_context(tc.tile_pool(name="mmps", bufs=MM_BUFS, space="PSUM"))
        st_ps = ctx.enter_context(tc.tile_pool(name="stps", bufs=1, space="PSUM"))
        sc_ps = ctx.enter_context(tc.tile_pool(name="scps", bufs=1, space="PSUM"))

        u_sb = const.tile([P, KT], f32)
        nc.sync.dma_start(u_sb, usum[0, :].rearrange("(m p) -> p m", p=P))
        ones128 = const.tile([P, 1], f32)
        nc.vector.memset(ones128, 1.0)
        ones2 = const.tile([2, 1], f32)
        nc.vector.memset(ones2, 1.0)
        w2p_sb = const.tile([P, 2 * KT], f32r)
        nc.sync.dma_start(w2p_sb, w2hl[0, :].rearrange("(c p) -> p c", p=P))

        def load_w1t(m, b):
            t = w1t_pool.tile([P, KT, P], f32r, tag="w1t", name=f"w1t_{b}_{m}")
            nc.sync.dma_start(t, w1tp[m].rearrange("(k p) i -> p k i", p=P))
            return t

        def tail_softmax(b, score2_ps):
            """Row-sum + transpose to [128, 16] via 16 K=2 matmuls; exp across
            128 lanes; global max broadcast via scalar DRAM bounce."""
            mx = sm_pool.tile([1, 1], f32, tag="mx", name=f"mx_{b}")
            nc.vector.reduce_max(mx, score2_ps[0:1, :], axis=mybir.AxisListType.X)
            nmx = sm_pool.tile([1, 1], f32, tag="nmx", name=f"nmx_{b}")
            nc.vector.tensor_scalar(
                nmx, mx, -1.0, -1.0, mybir.AluOpType.mult, mybir.AluOpType.add
            )
            scrm = dram_pool.tile([1, 1], f32, tag=f"scrm{b}", name=f"scrm_{b}")
            nc.sync.dma_start(scrm, nmx)
            nmx_bc = sm_pool.tile([P, 1], f32, tag="nmxbc", name=f"nmxbc_{b}")
            nc.sync.dma_start(nmx_bc, scrm.to_broadcast((P, 1)))

            s2 = sm_pool.tile([2, SEQ], f32, tag="s2", name=f"s2_{b}")
            nc.scalar.activation(s2, score2_ps, AF.Copy)
            scoret_ps = st_ps.tile([P, KT], f32, tag="st", name=f"st_{b}")
            for c in range(KT):
                nc.tensor.matmul(
                    scoret_ps[:, c : c + 1],
                    s2[0:2, c * P : (c + 1) * P],
                    ones2,
                    start=True,
                    stop=True,
                )
            expt = at_pool.tile([P, KT], f32r, tag=f"expt{b}", name=f"expt_{b}")
            sumv = sm_pool.tile([P, 1], f32, tag="sumv", name=f"sumv_{b}")
            nc.scalar.activation(expt, scoret_ps, AF.Exp, bias=nmx_bc, accum_out=sumv)
            z_ps_full = mm_ps.tile([P, 512], f32, tag="mm", name=f"zps_{b}")
            nc.tensor.matmul(
                z_ps_full[0:1, 0:1], ones128, sumv, start=True, stop=True
            )
            rcp = sm_pool.tile([1, 1], f32, tag=f"rcp{b}", name=f"rcp_{b}")
            nc.vector.reciprocal(rcp, z_ps_full[0:1, 0:1])
            return expt, rcp

        def tail_final(b, e_sb, expt, rcp):
            outsb = out_pool.tile([1, HIDDEN], f32, tag="outsb", name=f"outsb_{b}")
            for hc in range(2):
                fin_full = mm_ps.tile([P, 512], f32, tag="mm", name=f"fin_{b}_{hc}")
                fin = fin_full[0:1, :]
                hcol = slice(hc * 512, (hc + 1) * 512)
                for k in range(KT):
                    nc.tensor.matmul(
                        fin, expt[:, k : k + 1], e_sb[:, k, hcol],
                        start=(k == 0), stop=(k == KT - 1),
                    )
                nc.scalar.activation(outsb[:, hcol], fin, AF.Copy, scale=rcp)
            nc.sync.dma_start(out[b : b + 1, :], outsb)

        def load_dec(b):
            dec_bc = dec_pool.tile([P, HIDDEN], f32, tag="dec", name=f"dec_{b}")
            nc.sync.dma_start(dec_bc, dec[b : b + 1, :].to_broadcast((P, HIDDEN)))
            return dec_bc

        def main_loop(b, e_sb, w1t_first, dec_bc):
            score2_ps = sc_ps.tile([2, SEQ], f32, tag="score", name=f"score_{b}")
            # Right half first: tanh(u x dec) needs no E -- fills the E-load window
            for m in range(KT):
                whr = whr_pool.tile([P, HIDDEN], f32r, tag="whr", name=f"whr_{b}_{m}")
                nc.scalar.activation(whr, dec_bc, AF.Tanh, scale=u_sb[:, m : m + 1])
                for sc in (2, 3):
                    nc.tensor.matmul(
                        score2_ps[0:2, sc * 512 : (sc + 1) * 512],
                        w2p_sb[:, 2 * m : 2 * m + 2],
                        whr[:, (sc - 2) * 512 : (sc - 1) * 512],
                        start=(m == 0),
                        stop=(m == KT - 1),
                    )
            # Left half: the real matmul
            w1t = w1t_first
            for m in range(KT):
                w1t_next = load_w1t(m + 1, b) if m + 1 < KT else None
                whl = whl_pool.tile([P, HIDDEN], f32r, tag="whl", name=f"whl_{b}_{m}")
                for sc in range(2):
                    mm = mm_ps.tile([P, 512], f32, tag="mm", name=f"mm_{b}_{m}_{sc}")
                    ecol = slice(sc * 512, (sc + 1) * 512)
                    for k in range(KT):
                        nc.tensor.matmul(
                            mm, w1t[:, k, :], e_sb[:, k, ecol],
                            start=(k == 0), stop=(k == KT - 1),
                        )
                    nc.scalar.activation(whl[:, ecol], mm, AF.Tanh)
                for sc in range(2):
                    nc.tensor.matmul(
                        score2_ps[0:2, sc * 512 : (sc + 1) * 512],
                        w2p_sb[:, 2 * m : 2 * m + 2],
                        whl[:, sc * 512 : (sc + 1) * 512],
                        start=(m == 0),
                        stop=(m == KT - 1),
                    )
                w1t = w1t_next
            return score2_ps

        def load_e(b):
            e_sb = e_pool.tile([P, KT, HIDDEN], f32r, tag="e", name=f"e_{b}")
            enc_t = enc[b].rearrange("(k p) s -> p k s", p=P)
            for kg in range(0, KT, E_CHUNK):
                nc.sync.dma_start(
                    e_sb[:, kg : kg + E_CHUNK, :], enc_t[:, kg : kg + E_CHUNK, :]
                )
            return e_sb

        # software-pipelined schedule: w1t[0] first so PE starts ASAP;
        # batch b's final matmuls emitted after batch b+1's main loop so they
        # fill the PE gap during b+1's softmax chain.
        dec0 = load_dec(0)
        dec1 = load_dec(1)
        w1t0_b0 = load_w1t(0, 0)
        e0 = load_e(0)
        score0 = main_loop(0, e0, w1t0_b0, dec0)
        expt0, rcp0 = tail_softmax(0, score0)
        w1t0_b1 = load_w1t(0, 1)
        e1 = load_e(1)
        score1 = main_loop(1, e1, w1t0_b1, dec1)
        tail_final(0, e0, expt0, rcp0)
        expt1, rcp1 = tail_softmax(1, score1)
        tail_final(1, e1, expt1, rcp1)

    nc.compile()
    return nc


def _host_prep_opt(encoded_outputs, decoder_output, W1, w2):
    import ml_dtypes

    bf16 = ml_dtypes.bfloat16
    enc = np.ascontiguousarray(encoded_outputs, dtype=np.float32)
    dec = np.ascontiguousarray(decoder_output, dtype=np.float32)
    w1 = np.asarray(W1, dtype=np.float32)
    w2f = np.asarray(w2, dtype=np.float32).reshape(-1)

    a = np.ascontiguousarray(w1.T)                       # [t, i]
    w1tp = np.ascontiguousarray(a.reshape(SEQ, KT, P).transpose(1, 0, 2))  # [m, t, 128]
    usum = np.ascontiguousarray(w1.sum(axis=1).reshape(1, SEQ))
    w2hi = w2f.astype(bf16).astype(np.float32)           # exactly representable
    w2lo = w2f - w2hi
    # interleave hi/lo per 128-chunk: w2hl[(2m+j)*128+p] = (hi,lo)[j][m*128+p]
    pair = np.stack([w2hi.reshape(KT, P), w2lo.reshape(KT, P)], axis=1)  # [m, 2, p]
    w2hl = np.ascontiguousarray(pair.reshape(1, 2 * SEQ))
    return enc, dec, w1tp, usum, w2hl


def _build_nc_v2(whr_dt_name: str = "float32r"):
    """v2: PE runs (nearly) only the irreducible W1@E matmul rows.

    Differences vs opt:
      * score matmuls use the stationary/moving swap: stationary = wh block
        [128 i, 128 s], moving = w2 column [128 i, 1] -> out [128 s-part, 1].
        16 accumulating 1-col matmuls per s-block instead of streaming wh
        (32768 rows/batch -> ~0 PE rows). Plain f32 (exact, 1-col legal).
      * final matmuls likewise: stationary = E block [128 t, 128 h], moving =
        attn column (f32r needs even free dim -> zero-interleaved pairs).
        Output lands in h-partition layout [128, 8].
      * score lives in partition layout [128, 16] from the start; softmax
        runs there (cross-partition max/sum via PE transpose + ones-matmul
        broadcasts). No DRAM bounce.
      * batch0's right half (rank-1, needs no E) uses 512-row matmuls to fill
        the PE while E(b0) streams in; batch1's right half uses 1-col matmuls
        hidden under batch0's main loop.
    """
    from contextlib import ExitStack

    import concourse.tile as tile
    from concourse import bacc, mybir
    from concourse.masks import make_identity

    f32 = mybir.dt.float32
    f32r = mybir.dt.float32r
    whr_dt = getattr(mybir.dt, whr_dt_name)
    AF = mybir.ActivationFunctionType
    AX = mybir.AxisListType.X

    nc = bacc.Bacc("TRN2", target_bir_lowering=False, debug=False)

    # w1tp[m, t, :] = W1[m*128:(m+1)*128, t]^T  (W1^T blocks, host-prepped)
    w1tp = nc.dram_tensor("w1tp", [KT, SEQ, P], f32r, kind="ExternalInput").ap()
    enc = nc.dram_tensor("enc", [B_LOC, SEQ, HIDDEN], f32r, kind="ExternalInput").ap()
    dec = nc.dram_tensor("dec", [B_LOC, HIDDEN], f32, kind="ExternalInput").ap()
    usum = nc.dram_tensor("usum", [1, SEQ], f32, kind="ExternalInput").ap()
    w2p = nc.dram_tensor("w2p", [1, SEQ], f32, kind="ExternalInput").ap()
    out = nc.dram_tensor("out", [B_LOC, HIDDEN], f32, kind="ExternalOutput").ap()

    with tile.TileContext(nc) as tc, ExitStack() as ctx:
        const = ctx.enter_context(tc.tile_pool(name="const", bufs=1))
        e_pool = ctx.enter_context(tc.tile_pool(name="e", bufs=2))
        w1t_pool = ctx.enter_context(tc.tile_pool(name="w1t", bufs=W1T_BUFS))
        whl_pool = ctx.enter_context(tc.tile_pool(name="whl", bufs=2))
        whr_pool = ctx.enter_context(tc.tile_pool(name="whr", bufs=2))
        dec_pool = ctx.enter_context(tc.tile_pool(name="decp", bufs=2))
        sm_pool = ctx.enter_context(tc.tile_pool(name="sm", bufs=1))
        at_pool = ctx.enter_context(tc.tile_pool(name="at", bufs=2))
        out_pool = ctx.enter_context(tc.tile_pool(name="outp", bufs=2))

        # PSUM budget is 8 banks: mm 2 + scm 2 + rt 2 + small 1 + fin 1.
        # NOTE: only ONE matmul accumulation group may be live per bank at a
        # time; score sums therefore go through one-shot matmuls into scm
        # slots, accumulated into SBUF by the (otherwise idle) DVE.
        mm_ps = ctx.enter_context(tc.tile_pool(name="mmps", bufs=2, space="PSUM"))
        scm_ps = ctx.enter_context(tc.tile_pool(name="scmps", bufs=2, space="PSUM"))
        st_ps = ctx.enter_context(tc.tile_pool(name="stps", bufs=1, space="PSUM"))
        fin_ps = ctx.enter_context(tc.tile_pool(name="finps", bufs=1, space="PSUM"))

        # ---- constants
        u_sb = const.tile([P, KT], f32)
        nc.sync.dma_start(u_sb, usum[0, :].rearrange("(m p) -> p m", p=P))
        w2_sb = const.tile([P, KT], f32)  # w2 column per i-block, f32 exact
        nc.sync.dma_start(w2_sb, w2p[0, :].rearrange("(m p) -> p m", p=P))
        identity = const.tile([P, P], f32)
        make_identity(nc, identity)
        ones1 = const.tile([1, 1], f32)
        nc.vector.memset(ones1, 1.0)
        onesb = const.tile([1, P], f32)  # stationary for scalar->128 broadcast
        nc.vector.memset(onesb, 1.0)
        ones128 = const.tile([P, 1], f32)  # stationary for partition-sum
        nc.vector.memset(ones128, 1.0)
        zcol = const.tile([P, KT], f32)  # zeros for expt2's odd columns
        nc.vector.memset(zcol, 0.0)
        # [w2, 0] pairs for the 512-row whr matmuls (f32r stationary needs
        # even free dim; row 1 of the psum output is junk)
        w2z_sb = const.tile([P, 2 * KT], f32r)
        nc.vector.tensor_copy(w2z_sb[:, 0 : 2 * KT : 2], w2_sb)
        nc.vector.tensor_copy(w2z_sb[:, 1 : 2 * KT : 2], zcol)
        # SBUF score accumulators (PSUM can't hold 16 concurrently-live
        # accumulation groups in one bank)
        sacc0 = const.tile([P, KT], f32)
        nc.vector.memset(sacc0, 0.0)
        sacc1 = const.tile([P, KT], f32)
        nc.vector.memset(sacc1, 0.0)

        # ---- DMA helpers (all big loads on the sync queue; FIFO order matters)
        def load_dec(b):
            dec_bc = dec_pool.tile([P, HIDDEN], f32, tag="dec", name=f"dec_{b}")
            nc.sync.dma_start(dec_bc, dec[b : b + 1, :].to_broadcast((P, HIDDEN)))
            return dec_bc

        def load_w1t(b, m):
            t = w1t_pool.tile([P, KT, P], f32r, tag="w1t", name=f"w1t_{b}_{m}")
            nc.sync.dma_start(t, w1tp[m].rearrange("(k p) i -> p k i", p=P))
            return t

        def load_e(b):
            # chunk order matches m=0's consumption: sc-half major, k-group minor
            e_sb = e_pool.tile([P, KT, HIDDEN], f32r, tag="e", name=f"e_{b}")
            enc_t = enc[b].rearrange("(k p) s -> p k s", p=P)
            for sc in range(2):
                scs = slice(sc * 512, (sc + 1) * 512)
                for kg in range(0, KT, E_CHUNK):
                    nc.sync.dma_start(
                        e_sb[:, kg : kg + E_CHUNK, scs], enc_t[:, kg : kg + E_CHUNK, scs]
                    )
            return e_sb

        # ---- score helpers: 8 one-shot 1-col matmuls into a scratch PSUM
        # slot, then one DVE add into the SBUF accumulator.
        def emit_score_cols(b, m, wh, sacc, half, tag):
            scm = scm_ps.tile([P, 8], f32, tag="scm", name=f"scm_{tag}_{b}_{m}")
            for sb in range(8):
                nc.tensor.matmul(
                    scm[:, sb : sb + 1],
                    wh[:, sb * P : (sb + 1) * P],
                    w2_sb[:, m : m + 1],
                    start=True,
                    stop=True,
                )
            dst = sacc[:, half * 8 : half * 8 + 8]
            nc.vector.tensor_tensor(dst, dst, scm, mybir.AluOpType.add)

        def whr_head_512(b, dec_bc):
            """Right-half score for batch b via 512-row matmuls (PE filler while
            E(b) streams in). Output row 0: [1, 1024] in s'-free layout."""
            rt_ps = st_ps.tile([2, HIDDEN], f32, tag="rt", name=f"rt_{b}")
            for m in range(KT):
                whr = whr_pool.tile([P, HIDDEN], whr_dt, tag="whr", name=f"whr_{b}_{m}")
                nc.scalar.activation(whr, dec_bc, AF.Tanh, scale=u_sb[:, m : m + 1])
                for h in range(2):
                    nc.tensor.matmul(
                        rt_ps[0:2, h * 512 : (h + 1) * 512],
                        w2z_sb[:, 2 * m : 2 * m + 2],
                        whr[:, h * 512 : (h + 1) * 512],
                        start=(m == 0),
                        stop=(m == KT - 1),
                    )
            return rt_ps

        def rt_transpose(b, rt_ps, sacc):
            # [1, 1024] free layout -> sacc cols 8..15 (partition layout)
            s1 = sm_pool.tile([1, HIDDEN], f32, tag="s1", name=f"s1_{b}")
            nc.scalar.activation(s1, rt_ps[0:1, :], AF.Copy)
            scm = scm_ps.tile([P, 8], f32, tag="scm", name=f"scm_rt_{b}")
            for sb in range(8):
                nc.tensor.matmul(
                    scm[:, sb : sb + 1],
                    s1[0:1, sb * P : (sb + 1) * P],
                    ones1,
                    start=True,
                    stop=True,
                )
            dst = sacc[:, 8:16]
            nc.vector.tensor_tensor(dst, dst, scm, mybir.AluOpType.add)

        # ---- main loop
        def main_loop(b, e_sb, w1t0, sacc, post_iter=None):
            w1t = w1t0
            whl_prev = None
            for m in range(KT):
                w1t_next = load_w1t(b, m + 1) if m + 1 < KT else None
                whl = whl_pool.tile([P, HIDDEN], f32, tag="whl", name=f"whl_{b}_{m}")
                for sc in range(2):
                    mm = mm_ps.tile([P, 512], f32, tag="mm", name=f"mm_{b}_{m}_{sc}")
                    ecol = slice(sc * 512, (sc + 1) * 512)
                    for k in range(KT):
                        nc.tensor.matmul(
                            mm, w1t[:, k, :], e_sb[:, k, ecol],
                            start=(k == 0), stop=(k == KT - 1),
                        )
                    nc.scalar.activation(whl[:, ecol], mm, AF.Tanh)
                if whl_prev is not None:
                    emit_score_cols(b, m - 1, whl_prev, sacc, 0, "l")
                if post_iter is not None:
                    post_iter(m)
                whl_prev = whl
                w1t = w1t_next
            emit_score_cols(b, KT - 1, whl_prev, sacc, 0, "l")

        # ---- softmax in partition layout
        def tail_softmax(b, score_pt):
            # all small PSUM intermediates live in one shared bank
            small = st_ps.tile([P, 512], f32, tag="small", name=f"small_{b}")
            mxt = small[0:1, 0:P]
            nmb_ps = small[:, P : P + 1]
            z_ps = small[0:1, P + 1 : P + 2]
            rcb_ps = small[:, P + 2 : P + 3]
            mx = sm_pool.tile([P, 1], f32, tag="mx", name=f"mx_{b}")
            nc.vector.reduce_max(mx, score_pt, axis=AX)
            nc.tensor.transpose(mxt, mx, identity)
            gmx = sm_pool.tile([1, 1], f32, tag="gmx", name=f"gmx_{b}")
            nc.vector.reduce_max(gmx, mxt, axis=AX)
            nmx = sm_pool.tile([1, 1], f32, tag="nmx", name=f"nmx_{b}")
            nc.vector.tensor_scalar_mul(nmx, gmx, -1.0)
            nc.tensor.matmul(nmb_ps, onesb, nmx, start=True, stop=True)
            nmx_bc = sm_pool.tile([P, 1], f32, tag="nmxbc", name=f"nmxbc_{b}")
            nc.vector.tensor_copy(nmx_bc, nmb_ps)
            expt = at_pool.tile([P, KT], f32r, tag="expt", name=f"expt_{b}")
            sumv = sm_pool.tile([P, 1], f32, tag="sumv", name=f"sumv_{b}")
            nc.scalar.activation(expt, score_pt, AF.Exp, bias=nmx_bc, accum_out=sumv)
            # zero-interleaved copy (f32r matmul needs even moving free dim)
            expt2 = at_pool.tile([P, 2 * KT], f32r, tag="expt2", name=f"expt2_{b}")
            nc.vector.tensor_copy(expt2[:, 1 : 2 * KT : 2], zcol)
            nc.vector.tensor_copy(expt2[:, 0 : 2 * KT : 2], expt)
            nc.tensor.matmul(z_ps, ones128, sumv, start=True, stop=True)
            rcp = sm_pool.tile([1, 1], f32, tag="rcp", name=f"rcp_{b}")
            nc.vector.reciprocal(rcp, z_ps)
            nc.tensor.matmul(rcb_ps, onesb, rcp, start=True, stop=True)
            rcp_bc = sm_pool.tile([P, 1], f32, tag="rcpbc", name=f"rcpbc_{b}")
            nc.vector.tensor_copy(rcp_bc, rcb_ps)
            return expt2, rcp_bc

        def tail_final(b, e_sb, expt2, rcp_bc):
            outp = fin_ps.tile([P, 2 * 8], f32, tag="fin", name=f"fin_{b}")
            for hb in range(8):
                for k in range(KT):
                    nc.tensor.matmul(
                        outp[:, 2 * hb : 2 * hb + 2],
                        e_sb[:, k, hb * P : (hb + 1) * P],
                        expt2[:, 2 * k : 2 * k + 2],
                        start=(k == 0),
                        stop=(k == KT - 1),
                    )
            outsb = out_pool.tile([P, 8], f32, tag="outsb", name=f"outsb_{b}")
            nc.scalar.activation(outsb, outp[:, 0 : 2 * 8 : 2], AF.Copy, scale=rcp_bc)
            nc.sync.dma_start(out[b].rearrange("(hb p) -> p hb", p=P), outsb)

        # ================= schedule =================
        dec0 = load_dec(0)
        w1t0_b0 = load_w1t(0, 0)
        e0 = load_e(0)

        # head: b0 right-half fills PE while E(b0) streams
        rt0 = whr_head_512(0, dec0)
        rt_transpose(0, rt0, sacc0)
        dec1 = load_dec(1)  # FIFO: after E(b0)

        # b0 main loop; b1's right half (1-col matmuls) rides along, one m per iter
        whr1_tiles = {}

        def post_iter_b0(m):
            whr = whr_pool.tile([P, HIDDEN], f32, tag="whr1", name=f"whr1_{m}")
            nc.scalar.activation(whr, dec1, AF.Tanh, scale=u_sb[:, m : m + 1])
            whr1_tiles[m] = whr
            if m - 1 in whr1_tiles:
                emit_score_cols(1, m - 1, whr1_tiles.pop(m - 1), sacc1, 1, "r")

        main_loop(0, e0, w1t0_b0, sacc0, post_iter=post_iter_b0)
        emit_score_cols(1, KT - 1, whr1_tiles.pop(KT - 1), sacc1, 1, "r")

        # FIFO: E(b1) + its first W1T block queue behind b0's w1t stream
        e1 = load_e(1)
        w1t0_b1 = load_w1t(1, 0)

        tail_state = {}

        def post_iter_b1(m):
            if m == 0:
                tail_state["sm0"] = tail_softmax(0, sacc0)
            elif m == 1:
                expt2, rcp_bc = tail_state.pop("sm0")
                tail_final(0, e0, expt2, rcp_bc)

        main_loop(1, e1, w1t0_b1, sacc1, post_iter=post_iter_b1)
        expt2_1, rcp_bc_1 = tail_softmax(1, sacc1)
        tail_final(1, e1, expt2_1, rcp_bc_1)

    nc.compile()
    return nc


def _host_prep_v2(encoded_outputs, decoder_output, W1, w2):
    enc = np.ascontiguousarray(encoded_outputs, dtype=np.float32)
    dec = np.ascontiguousarray(decoder_output, dtype=np.float32)
    w1 = np.asarray(W1, dtype=np.float32)
    w2f = np.ascontiguousarray(np.asarray(w2, dtype=np.float32).reshape(1, SEQ))

    a = np.ascontiguousarray(w1.T)                       # [t, i]
    w1tp = np.ascontiguousarray(a.reshape(SEQ, KT, P).transpose(1, 0, 2))  # [m, t, 128]
    usum = np.ascontiguousarray(w1.sum(axis=1).reshape(1, SEQ))
    return enc, dec, w1tp, usum, w2f


def _get_nc(mode: str):
    if mode not in _CACHE:
        if mode == "bf16x3":
            _CACHE[mode] = _build_nc_bf16x3()
        elif mode == "opt":
            _CACHE[mode] = _build_nc_opt()
        elif mode == "v2":
            _CACHE[mode] = _build_nc_v2()
        elif mode == "v2_f32whr":
            _CACHE[mode] = _build_nc_v2("float32")
        else:
            _CACHE[mode] = _build_nc_legacy(mode)
    return _CACHE[mode]


MM_DTYPE = "v2"


def kernel(encoded_outputs, decoder_output, W1, w2):
    from concourse.bass_utils import run_bass_kernel_spmd

    nc = _get_nc(MM_DTYPE)
    if MM_DTYPE.startswith("v2"):
        enc, dec, w1tp, usum, w2f = _host_prep_v2(
            encoded_outputs, decoder_output, W1, w2
        )
        in_maps = [
            {
                "enc": np.ascontiguousarray(enc[i * B_LOC : (i + 1) * B_LOC]),
                "dec": np.ascontiguousarray(dec[i * B_LOC : (i + 1) * B_LOC]),
                "w1tp": w1tp,
                "usum": usum,
                "w2p": w2f,
            }
            for i in range(N_CORES)
        ]
    elif MM_DTYPE == "opt":
        enc, dec, w1tp, usum, w2hl = _host_prep_opt(
            encoded_outputs, decoder_output, W1, w2
        )
        in_maps = [
            {
                "enc": np.ascontiguousarray(enc[i * B_LOC : (i + 1) * B_LOC]),
                "dec": np.ascontiguousarray(dec[i * B_LOC : (i + 1) * B_LOC]),
                "w1tp": w1tp,
                "usum": usum,
                "w2hl": w2hl,
            }
            for i in range(N_CORES)
        ]
    elif MM_DTYPE == "bf16x3":
        enc, dec, w2c, w1tp, e_hi, e_lo, usum = _host_prep(
            encoded_outputs, decoder_output, W1, w2
        )
        in_maps = [
            {
                "ehi": np.ascontiguousarray(e_hi[i * B_LOC : (i + 1) * B_LOC]),
                "elo": np.ascontiguousarray(e_lo[i * B_LOC : (i + 1) * B_LOC]),
                "dec": np.ascontiguousarray(dec[i * B_LOC : (i + 1) * B_LOC]),
                "w1tp": w1tp,
                "usum": usum,
                "w2": w2c,
            }
            for i in range(N_CORES)
        ]
    else:
        enc = np.ascontiguousarray(encoded_outputs, dtype=np.float32)
        dec = np.ascontiguousarray(decoder_output, dtype=np.float32)
        w1 = np.ascontiguousarray(W1, dtype=np.float32)
        w2c = np.ascontiguousarray(w2, dtype=np.float32)
        in_maps = [
            {
                "enc": np.ascontiguousarray(enc[i * B_LOC : (i + 1) * B_LOC]),
                "dec": np.ascontiguousarray(dec[i * B_LOC : (i + 1) * B_LOC]),
                "w1": w1,
                "w2": w2c,
            }
            for i in range(N_CORES)
        ]
    res = run_bass_kernel_spmd(nc, in_maps, core_ids=list(range(N_CORES)))
    return np.concatenate([r["out"] for r in res.results], axis=0)

